# revision 67
# baseline (speedup 1.0000x reference)
"""Trainium2 Bass kernel for nn_AttentionBiasHead (B=16, L=2048, DIN=1024, DQ=128).

Sharding: data-parallel over batch -- 16 batches -> 8 cores x 2 batches each.
Host pre-transposes activations to [DIN, L] layout and casts to bf16.

Per batch, on-device (all layouts transposed: feature dim on partitions):
  qT/kT/vT = W^T @ xT + b           [128, L]  (ScalarE applies per-partition bias)
  cpeT     = MLP(sf, pos)           [128, L]  (pos @ W1 part precomputed per core)
  qcT/kcT  = Wc^T @ cpeT + bc       [128, L]
  S^T tiles = kT_m^T @ qT  -> exp -> E^T bf16.  No max-subtraction needed:
      |scores|/sqrt(dq) < ~2.5 so exp is safe in fp32/bf16.
  out = softmax@v + bias@v with bias = qc kc^T never materialized:
      softmax@v: per l-tile accumulate E^T_m[:,l]^T @ [v_m | 1] -> unnorm + rowsum
                 (ones column of the rhs yields softmax denominators for free)
      bias@v   = qc @ (kc^T v)   (kc^T v = sum_m kc_nat_m^T @ v_m, rank-128)
  norm3 L1 diff terms: the row/col difference matrices of bias are computed
      directly on the TensorEngine:  D_r^T = kc @ dqc^T,  D_c^T = dkc @ qc^T
      (dqc/dkc = adjacent diffs of qc/kc along L, biases cancel) and reduced
      with fused |.|+sum ops: ScalarE activation(Abs, accum_out=...) and
      VectorE tensor_reduce(apply_absolute_value=True), split to balance load.
"""

import math
import sys

import numpy as np

if "/opt/trn_rl_repo" not in sys.path:
    sys.path.insert(0, "/opt/trn_rl_repo")

import ml_dtypes

BF16 = ml_dtypes.bfloat16

P = 128
L = 2048
DIN = 1024
KD = DIN // P  # 8 contraction chunks for the projections
LT = L // P    # 16 l-tiles
MT = L // P    # 16 m-tiles
NB = 2         # batches per core
NCORES = 8
SCALE = 1.0 / math.sqrt(128.0)

# Fraction of the norm3 abs-sum reduces sent to ScalarE (rest go to VectorE).
ACT_REDUCE_EVERY = 16   # of each 16 reduces, first ACT_REDUCE_TAKE go to ScalarE
ACT_REDUCE_TAKE = 1
ACT_SPLIT_AT = 80       # quarters before this index use the EARLY split
ACT_EARLY_EVERY = 5
ACT_EARLY_TAKE = 2
N_LATE = 0              # last D_c m-tiles deferred until after S^T (pa halves)

_cache = {}
LAST_RESULTS = None  # BassKernelResults of the most recent run (for test.py)
TRACE = False


def _emit(tc, ctx, io, reps=1):
    import concourse.mybir as mybir
    from concourse.bass import ts
    from concourse.masks import make_identity

    nc = tc.nc
    f32 = mybir.dt.float32
    bf16 = mybir.dt.bfloat16
    Alu = mybir.AluOpType
    Act = mybir.ActivationFunctionType
    AX = mybir.AxisListType.X

    consts = ctx.enter_context(tc.tile_pool(name="consts", bufs=1))
    xin = ctx.enter_context(tc.tile_pool(name="xin", bufs=6))
    state = ctx.enter_context(tc.tile_pool(name="state", bufs=2))
    epool = ctx.enter_context(tc.tile_pool(name="epool", bufs=1))
    opool = ctx.enter_context(tc.tile_pool(name="opool", bufs=4))
    # PSUM budget (8 banks): pa = proj/S^T/cpe [128,1024]x2 -> 4 banks,
    # ps = shared small slots ([128,<=512] f32, one bank each) x4 -> 4 banks.
    pa = ctx.enter_context(tc.tile_pool(name="pa", bufs=2, space="PSUM"))
    psm = ctx.enter_context(tc.tile_pool(name="psm", bufs=4, space="PSUM"))

    # ---------------- constants ----------------
    # Small tensors first: they gate the CPE/norm3 chain that fills the
    # startup window; the bulky projection weights follow.
    post = consts.tile([P, L], bf16, tag="post", name="post_sb")
    nc.sync.dma_start(post[:], io["post"][:])
    w1pos = consts.tile([P, P], bf16, tag="w1pos", name="w1pos_sb")
    nc.sync.dma_start(w1pos[:], io["w1pos"][:])
    w1sf = consts.tile([P, P], bf16, tag="w1sf", name="w1sf_sb")
    nc.sync.dma_start(w1sf[:], io["w1sf"][:])
    sfc = consts.tile([P, NB], bf16, tag="sfc", name="sfc_sb")
    nc.sync.dma_start(sfc[:], io["sfc"][:])
    bias_pack = consts.tile([P, 8], f32, tag="bias_pack", name="bias_pack_sb")
    nc.sync.dma_start(bias_pack[:], io["bias_pack"][:])
    w2 = consts.tile([P, P], bf16, tag="w2", name="w2_sb")
    nc.sync.dma_start(w2[:], io["w2"][:])
    wqc = consts.tile([P, P], bf16, tag="wqc", name="wqc_sb")
    nc.sync.dma_start(wqc[:], io["wqc"][:])
    wkc = consts.tile([P, P], bf16, tag="wkc", name="wkc_sb")
    nc.sync.dma_start(wkc[:], io["wkc"][:])
    bkc_pad = consts.tile([P, P], bf16, tag="bkc_pad", name="bkc_pad_sb")
    nc.sync.dma_start(bkc_pad[:], io["bkc_pad"][:])
    wq = consts.tile([P, KD, P], bf16, tag="wq", name="wq_sb")
    wk = consts.tile([P, KD, P], bf16, tag="wk", name="wk_sb")
    wv = consts.tile([P, KD, P], bf16, tag="wv", name="wv_sb")
    for t, nm in ((wq, "wq"), (wk, "wk"), (wv, "wv")):
        nc.sync.dma_start(t[:], io[nm][:])

    bq = bias_pack[:, 0:1]
    bk = bias_pack[:, 1:2]
    bv = bias_pack[:, 2:3]
    b1 = bias_pack[:, 3:4]
    b2 = bias_pack[:, 4:5]
    bqc = bias_pack[:, 5:6]
    bkc = bias_pack[:, 6:7]

    ident = consts.tile([P, P], bf16, tag="ident", name="ident_sb")
    make_identity(nc, ident)
    ones_col = consts.tile([P, 1], f32, tag="ones_col", name="ones_col_sb")
    nc.vector.memset(ones_col[:], 1.0)
    ones_col_bf = consts.tile([P, 1], bf16, tag="ones_col_bf", name="ones_col_bf_sb")
    nc.vector.memset(ones_col_bf[:], 1.0)

    # posW1T = W1pos^T @ posT  (batch independent, keep fp32)
    posw1 = consts.tile([P, L], f32, tag="posw1", name="posw1_sb")
    for lh in range(2):
        ps = pa.tile([P, 1024], f32, tag="pa", name="ps_posw1")
        for j in range(2):
            c0 = lh * 1024 + j * 512
            nc.tensor.matmul(
                ps[:, j * 512:(j + 1) * 512], w1pos[:], post[:, c0:c0 + 512],
                start=True, stop=True,
            )
        nc.scalar.activation(posw1[:, lh * 1024:(lh + 1) * 1024], ps[:], Act.Copy)

    n3sb = consts.tile([1, NB], f32, tag="n3sb", name="n3sb_sb")

    def phase_cpe(b, st):
        # sf contribution to the first-layer bias: sfb1 = W1sf^T sf_b + b1
        sfps = psm.tile([P, 1], f32, tag="sm", name="sfps")
        nc.tensor.matmul(sfps[:], w1sf[:], sfc[:, b:b + 1], start=True, stop=True)
        sfb1 = state.tile([P, 1], f32, tag="sfb1", name="sfb1")
        nc.vector.tensor_add(sfb1[:], sfps[:], b1)

        # CPE stack:  h1 = max(posW1T + sfb1, 0)  (fused on DVE)
        h1 = state.tile([P, L], bf16, tag="h1", name="h1")
        for lh in range(2):
            sl = slice(lh * 1024, (lh + 1) * 1024)
            nc.vector.tensor_scalar(
                h1[:, sl], posw1[:, sl], sfb1[:], 0.0,
                Alu.add, Alu.max,
            )

        def lin128(dst, w_sb, src, bcol):
            # psm quarters so the CPE stack never contends with proj/S^T psum
            for q in range(4):
                lp = psm.tile([P, 512], f32, tag="sm", name="ps_lin")
                nc.tensor.matmul(
                    lp[:], w_sb[:], src[:, q * 512:(q + 1) * 512],
                    start=True, stop=True,
                )
                nc.scalar.activation(
                    dst[:, q * 512:(q + 1) * 512], lp[:], Act.Identity, bias=bcol,
                )

        cpeT = state.tile([P, L], bf16, tag="cpeT", name="cpeT")
        lin128(cpeT, w2, h1, b2)
        qcT = state.tile([P, L], bf16, tag="qcT", name="qcT")
        lin128(qcT, wqc, cpeT, bqc)
        kcT = state.tile([P, L], bf16, tag="kcT", name="kcT")
        lin128(kcT, wkc, cpeT, bkc)

        dqcT = state.tile([P, L - 1], bf16, tag="dqcT", name="dqcT")
        nc.vector.tensor_sub(dqcT[:], qcT[:, 1:L], qcT[:, 0:L - 1])
        dkcT = state.tile([P, L - 1], bf16, tag="dkcT", name="dkcT")
        nc.vector.tensor_sub(dkcT[:], kcT[:, 1:L], kcT[:, 0:L - 1])
        st.update(cpeT=cpeT, qcT=qcT, kcT=kcT, dqcT=dqcT, dkcT=dkcT)

    def phase_norm3(b, st):
        # norm3: difference matrices in 512-wide quarters + fused abs-sum
        # reduces.  Emitted early so this PE/DVE/ACT work fills the DMA-bound
        # projection phases and the ACT-bound exp phases of both batches.
        if "acc" not in st:
            acc = state.tile([P, 128], f32, tag="acc", name="acc")
            nc.vector.memset(acc[:], 0.0)
            st["acc"] = acc
            st["ridx"] = 0
        acc = st["acc"]
        ridx = st["ridx"]

        def emit_diff_matrix(lhs_t, last_w, rhs_t, rhs_w, m_stop=MT):
            nonlocal ridx
            for m in range(m_stop):
                mw = P if m < MT - 1 else last_w
                for q in range(4):
                    w = min(512, rhs_w - q * 512)
                    dps = psm.tile([P, 512], f32, tag="sm", name="dps")
                    nc.tensor.matmul(
                        dps[:mw, :w],
                        lhs_t[:, m * P:m * P + mw], rhs_t[:, q * 512:q * 512 + w],
                        start=True, stop=True,
                    )
                    if (ridx % ACT_REDUCE_EVERY < ACT_REDUCE_TAKE
                            if ridx >= ACT_SPLIT_AT else
                            ridx % ACT_EARLY_EVERY < ACT_EARLY_TAKE):
                        junk = opool.tile([P, 512], bf16, tag="junk", name="junk")
                        nc.scalar.activation(
                            junk[:mw, :w], dps[:mw, :w], Act.Abs,
                            accum_out=acc[:mw, ridx:ridx + 1],
                        )
                    else:
                        nc.vector.tensor_reduce(
                            acc[:mw, ridx:ridx + 1], dps[:mw, :w],
                            axis=AX, op=Alu.add, apply_absolute_value=True,
                        )
                    ridx += 1

        emit_diff_matrix(st["kcT"], P, st["dqcT"], L - 1)   # D_r^T = kc dqc^T
        emit_diff_matrix(st["dkcT"], P - 1, st["qcT"], L,
                         m_stop=MT - N_LATE)                # D_c^T (most of it)

    def phase_main(b, st):
        cpeT, qcT, acc = st["cpeT"], st["qcT"], st["acc"]
        # ---- projections qT/kT/vT = W^T xT + b
        qT = state.tile([P, L], bf16, tag="qT", bufs=1, name="qT")
        kT = state.tile([P, L], bf16, tag="kT", bufs=1, name="kT")
        vT = state.tile([P, L], bf16, tag="vT", bufs=1, name="vT")
        for nm, wsb, bcol, dst in (
            ("xq", wq, bq, qT), ("xk", wk, bk, kT), ("xv", wv, bv, vT),
        ):
            pss = [pa.tile([P, 1024], f32, tag="pa", name=f"ps_{nm}{lh}") for lh in range(2)]
            for k in range(KD):
                for lh in range(2):
                    ch = xin.tile([P, 1024], bf16, tag=nm, name=f"ch_{nm}")
                    nc.sync.dma_start(
                        ch[:], io[nm][b, ts(k, P), lh * 1024:(lh + 1) * 1024])
                    for j in range(2):
                        nc.tensor.matmul(
                            pss[lh][:, j * 512:(j + 1) * 512],
                            wsb[:, k, :], ch[:, j * 512:(j + 1) * 512],
                            start=(k == 0), stop=(k == KD - 1),
                        )
            for lh in range(2):
                nc.scalar.activation(
                    dst[:, lh * 1024:(lh + 1) * 1024], pss[lh][:],
                    Act.Identity, bias=bcol,
                )

        # ---- v natural layout tiles [m, dq] with an appended ones column
        vall = state.tile([P, MT, 132], bf16, tag="vall", name="vall")
        nc.vector.memset(vall[:, :, 128:129], 1.0)
        for m in range(MT):
            tp = psm.tile([P, P], bf16, tag="sm", name="tp_v")
            nc.tensor.transpose(tp[:], vT[:, ts(m, P)], ident[:])
            nc.vector.tensor_copy(vall[:, m, 0:P], tp[:])

        # ---- kc natural tiles (no bias) and T = kc^T v
        #      T = kc0^T v + bkc (x) vsum   with vsum = 1^T v
        kcn = state.tile([P, MT, P], bf16, tag="kcn", bufs=1, name="kcn")
        for m in range(MT):
            kp = psm.tile([P, P], f32, tag="sm", name="kp")
            nc.tensor.matmul(kp[:], cpeT[:, ts(m, P)], wkc[:], start=True, stop=True)
            nc.vector.tensor_copy(kcn[:, m, :], kp[:])
        vsump = psm.tile([1, P], f32, tag="sm", name="vsump")
        for m in range(MT):
            nc.tensor.matmul(
                vsump[:], ones_col_bf[:], vall[:, m, 0:P],
                start=(m == 0), stop=(m == MT - 1),
            )
        vsum_pad = state.tile([P, P], bf16, tag="vsum_pad", name="vsum_pad")
        nc.vector.memset(vsum_pad[:], 0.0)
        nc.vector.tensor_copy(vsum_pad[0:1, :], vsump[:])
        tps = psm.tile([P, P], f32, tag="sm", name="tps")
        for m in range(MT):
            nc.tensor.matmul(
                tps[:], kcn[:, m, :], vall[:, m, 0:P],
                start=(m == 0), stop=False,
            )
        nc.tensor.matmul(tps[:], bkc_pad[:], vsum_pad[:], start=False, stop=True)
        tsb = state.tile([P, P], bf16, tag="tsb", name="tsb")
        nc.vector.tensor_copy(tsb[:], tps[:])

        # ---- S^T -> exp -> E^T and attention output, interleaved by l-half:
        # the E@v matmuls of one half run while the other half's exps occupy
        # ScalarE.
        for lh in range(2):
            ehalf = epool.tile([P, MT, L // 2], bf16, tag=f"e{lh}", name=f"e{lh}")
            for m in range(MT):
                sps = pa.tile([P, 1024], f32, tag="pa", name="sps")
                for j in range(2):
                    c0 = lh * 1024 + j * 512
                    nc.tensor.matmul(
                        sps[:, j * 512:(j + 1) * 512],
                        kT[:, ts(m, P)], qT[:, c0:c0 + 512],
                        start=True, stop=True,
                    )
                nc.scalar.activation(
                    ehalf[:, m, :], sps[:], Act.Exp,
                    scale=SCALE,
                )
            for l in range(lh * 8, lh * 8 + 8):
                av = psm.tile([P, 129], f32, tag="sm", name="av")
                for m in range(MT):
                    nc.tensor.matmul(
                        av[:], ehalf[:, m, ts(l % 8, P)], vall[:, m, 0:129],
                        start=(m == 0), stop=(m == MT - 1),
                    )
                av_sb = opool.tile([P, 129], f32, tag="av_sb", name="av_sb")
                nc.scalar.activation(av_sb[:], av[:], Act.Copy)
                bvp = psm.tile([P, P], f32, tag="sm", name="bvp")
                nc.tensor.matmul(bvp[:], qcT[:, ts(l, P)], tsb[:],
                                 start=True, stop=True)
                bvp_sb = opool.tile([P, P], f32, tag="bvp_sb", name="bvp_sb")
                nc.vector.tensor_copy(bvp_sb[:], bvp[:])
                rc = opool.tile([P, 1], f32, tag="rc", name="rc")
                nc.vector.reciprocal(rc[:], av_sb[:, 128:129])
                o2 = opool.tile([P, P], f32, tag="o2", name="o2")
                nc.vector.scalar_tensor_tensor(
                    o2[:], av_sb[:, 0:P], rc[:], bvp_sb[:], Alu.mult, Alu.add,
                )
                nc.sync.dma_start(io["out"][b, ts(l, P), :], o2[:])

        # ---- deferred D_c tail in the pa pool (idle after S^T): fills the
        # DVE-idle stretch of the E@v phase.
        ridx = st["ridx"]
        for m in range(MT - N_LATE, MT):
            mw = P if m < MT - 1 else P - 1
            for lh in range(2):
                dps = pa.tile([P, 1024], f32, tag="pa", name="dps_late")
                for j in range(2):
                    c0 = lh * 1024 + j * 512
                    nc.tensor.matmul(
                        dps[:mw, j * 512:(j + 1) * 512],
                        st["dkcT"][:, m * P:m * P + mw], qcT[:, c0:c0 + 512],
                        start=True, stop=True,
                    )
                nc.vector.tensor_reduce(
                    acc[:mw, ridx:ridx + 1], dps[:mw, :],
                    axis=AX, op=Alu.add, apply_absolute_value=True,
                )
                ridx += 1
        st["ridx"] = ridx

        # ---- norm3 finalize
        red = state.tile([P, 1], f32, tag="red", name="red")
        nc.vector.tensor_reduce(red[:], acc[:], axis=AX, op=Alu.add)
        n3ps = psm.tile([1, 1], f32, tag="sm", name="n3ps")
        nc.tensor.matmul(n3ps[:], red[:], ones_col[:], start=True, stop=True)
        nc.scalar.activation(n3sb[:, b:b + 1], n3ps[:], Act.Copy)

    occs = [bb % NB for bb in range(NB * reps)]
    sts = [{} for _ in occs]
    for b, st in zip(occs, sts):
        phase_cpe(b, st)
        phase_norm3(b, st)      # fills the DMA-bound proj + ACT-bound exp phases
        phase_main(b, st)

    nc.sync.dma_start(io["n3p"][:], n3sb[:])


def _get_module(reps=1):
    key = f"nc{reps}"
    if key in _cache:
        return _cache[key]
    from contextlib import ExitStack

    import concourse.mybir as mybir
    import concourse.tile as tile
    from concourse import bacc

    f32 = mybir.dt.float32
    bf16 = mybir.dt.bfloat16

    nc = bacc.Bacc(
        "TRN2", target_bir_lowering=False, debug=False,
        enable_asserts=False, enable_partition_id=False,
    )
    io = {
        "xq": nc.dram_tensor("xq", [NB, DIN, L], bf16, kind="ExternalInput")[:],
        "xk": nc.dram_tensor("xk", [NB, DIN, L], bf16, kind="ExternalInput")[:],
        "xv": nc.dram_tensor("xv", [NB, DIN, L], bf16, kind="ExternalInput")[:],
        "post": nc.dram_tensor("post", [P, L], bf16, kind="ExternalInput")[:],
        "sfc": nc.dram_tensor("sfc", [P, NB], bf16, kind="ExternalInput")[:],
        "wq": nc.dram_tensor("wq", [P, KD, P], bf16, kind="ExternalInput")[:],
        "wk": nc.dram_tensor("wk", [P, KD, P], bf16, kind="ExternalInput")[:],
        "wv": nc.dram_tensor("wv", [P, KD, P], bf16, kind="ExternalInput")[:],
        "w1sf": nc.dram_tensor("w1sf", [P, P], bf16, kind="ExternalInput")[:],
        "w1pos": nc.dram_tensor("w1pos", [P, P], bf16, kind="ExternalInput")[:],
        "w2": nc.dram_tensor("w2", [P, P], bf16, kind="ExternalInput")[:],
        "wqc": nc.dram_tensor("wqc", [P, P], bf16, kind="ExternalInput")[:],
        "wkc": nc.dram_tensor("wkc", [P, P], bf16, kind="ExternalInput")[:],
        "bias_pack": nc.dram_tensor("bias_pack", [P, 8], f32, kind="ExternalInput")[:],
        "bkc_pad": nc.dram_tensor("bkc_pad", [P, P], bf16, kind="ExternalInput")[:],
        "out": nc.dram_tensor("out", [NB, L, P], f32, kind="ExternalOutput")[:],
        "n3p": nc.dram_tensor("n3p", [1, NB], f32, kind="ExternalOutput")[:],
    }

    with tile.TileContext(nc) as tc:
        with ExitStack() as ctx:
            _emit(tc, ctx, io, reps=reps)
    nc.compile()

    _cache[key] = nc
    return nc


def _host_prep(query, key, value, sf, pos, Wq, bq, Wk, bk, Wv, bv,
               W1, b1, W2, b2, Wqc, bqc, Wkc, bkc):
    def f32a(x):
        return np.asarray(x, dtype=np.float32)

    xq = f32a(query).transpose(0, 2, 1).astype(BF16)  # [16, DIN, L]
    xk = f32a(key).transpose(0, 2, 1).astype(BF16)
    xv = f32a(value).transpose(0, 2, 1).astype(BF16)
    # pad the small contraction dims (32/64) up to 128 partitions with zeros
    post = np.zeros((P, L), dtype=BF16)
    post[:32] = f32a(pos).T.astype(BF16)
    sfT = np.zeros((P, 16), dtype=BF16)
    sfT[:64] = f32a(sf).T.astype(BF16)

    def wsplit(W):
        return f32a(W).reshape(KD, P, P).transpose(1, 0, 2).astype(BF16)

    bias_pack = np.zeros((P, 8), dtype=np.float32)
    for i, bb in enumerate((bq, bk, bv, b1, b2, bqc, bkc)):
        bias_pack[:, i] = f32a(bb)

    shared = {
        "post": post,
        "wq": wsplit(Wq), "wk": wsplit(Wk), "wv": wsplit(Wv),
        "w1sf": np.concatenate(
            [f32a(W1)[:64], np.zeros((64, P), np.float32)], 0).astype(BF16),
        "w1pos": np.concatenate(
            [f32a(W1)[64:], np.zeros((96, P), np.float32)], 0).astype(BF16),
        "w2": f32a(W2).astype(BF16),
        "wqc": f32a(Wqc).astype(BF16),
        "wkc": f32a(Wkc).astype(BF16),
        "bias_pack": bias_pack,
        "bkc_pad": np.concatenate(
            [f32a(bkc).reshape(1, P), np.zeros((127, P), np.float32)], 0).astype(BF16),
    }
    in_maps = []
    for c in range(NCORES):
        m = dict(shared)
        m["xq"] = np.ascontiguousarray(xq[NB * c:NB * (c + 1)])
        m["xk"] = np.ascontiguousarray(xk[NB * c:NB * (c + 1)])
        m["xv"] = np.ascontiguousarray(xv[NB * c:NB * (c + 1)])
        m["sfc"] = np.ascontiguousarray(sfT[:, NB * c:NB * (c + 1)])
        in_maps.append(m)
    return in_maps


def time_device(in_maps, chains=(1, 5), reps=4, module_reps=1):
    """Measure per-execution device time by chaining the NEFF body inside one
    jit: iteration i's outputs are fed as iteration i+1's (donated) output
    buffers, which serializes executions without host round-trips.  Returns
    min over reps of (t[c1] - t[c0]) / (c1 - c0)."""
    import time

    import jax
    import numpy as np
    from jax.sharding import Mesh, NamedSharding, PartitionSpec

    try:
        from jax.experimental.shard_map import shard_map
    except ImportError:
        from jax.shard_map import shard_map

    import concourse.mybir as mybir
    from concourse.bass2jax import _bass_exec_p, install_neuronx_cc_hook

    nc = _get_module(reps=module_reps)
    install_neuronx_cc_hook()

    in_names, out_names, out_avals, zero_outs = [], [], [], []
    for alloc in nc.m.functions[0].allocations:
        if not isinstance(alloc, mybir.MemoryLocationSet):
            continue
        name = alloc.memorylocations[0].name
        if alloc.kind == "ExternalInput":
            in_names.append(name)
        elif alloc.kind == "ExternalOutput":
            out_names.append(name)
            shape = tuple(alloc.tensor_shape)
            dtype = mybir.dt.np(alloc.dtype)
            out_avals.append(jax.core.ShapedArray(shape, dtype))
            zero_outs.append(np.zeros(shape, dtype))
    n_params = len(in_names)
    all_names = tuple(in_names + out_names)

    def make_chain(k):
        assert k == 1, "hook supports a single bass_exec per module"

        def _chain(*args):
            ins = args[:n_params]
            zs = tuple(args[n_params:])
            return tuple(_bass_exec_p.bind(
                *ins, *zs,
                out_avals=tuple(out_avals),
                in_names=all_names,
                out_names=tuple(out_names),
                lowering_input_output_aliases=(),
                sim_require_finite=True,
                sim_require_nnan=True,
                nc=nc,
            ))
        return _chain

    n = NCORES
    devices = jax.devices()[:n]
    mesh = Mesh(np.asarray(devices), ("core",))
    spec = PartitionSpec("core")
    n_args = n_params + len(out_names)

    concat_in = [
        np.concatenate([np.asarray(in_maps[c][nm]) for c in range(n)], axis=0)
        for nm in in_names
    ]
    concat_zeros = [np.zeros((n * z.shape[0], *z.shape[1:]), z.dtype)
                    for z in zero_outs]
    sh = NamedSharding(mesh, spec)
    dev_in = [jax.device_put(a, sh) for a in concat_in]
    dev_zeros = [jax.device_put(a, sh) for a in concat_zeros]

    fn = jax.jit(shard_map(make_chain(1), mesh=mesh,
                           in_specs=(spec,) * n_args,
                           out_specs=(spec,) * len(out_names),
                           check_rep=False))
    jax.block_until_ready(fn(*dev_in, *dev_zeros))  # warmup/compile

    # Async-pipelined batches: dispatch N executions without blocking, then
    # block once.  Marginal time per extra execution ~= device time.
    def run_batch(n):
        outs = []
        t0 = time.perf_counter()
        for _ in range(n):
            outs.append(fn(*dev_in, *dev_zeros))
        jax.block_until_ready(outs)
        return time.perf_counter() - t0

    run_batch(2)
    n_lo, n_hi = 4, 24
    t_lo = min(run_batch(n_lo) for _ in range(reps))
    t_hi = min(run_batch(n_hi) for _ in range(reps))
    print(f"  batch{n_lo}: {t_lo*1e6:.0f} us   batch{n_hi}: {t_hi*1e6:.0f} us")
    per = (t_hi - t_lo) / (n_hi - n_lo)
    return per * 1e9


def kernel(query, key, value, sf, pos, Wq, bq, Wk, bk, Wv, bv,
           W1, b1, W2, b2, Wqc, bqc, Wkc, bkc):
    global LAST_RESULTS
    from concourse import bass_utils

    nc = _get_module()
    in_maps = _host_prep(query, key, value, sf, pos, Wq, bq, Wk, bk, Wv, bv,
                         W1, b1, W2, b2, Wqc, bqc, Wkc, bkc)
    res = bass_utils.run_bass_kernel_spmd(
        nc, in_maps, core_ids=list(range(NCORES)), trace=TRACE,
    )
    LAST_RESULTS = res
    out = np.concatenate([np.asarray(r["out"]) for r in res.results], axis=0)
    out = out.astype(np.float32)
    norm3 = np.float32(
        sum(float(np.asarray(r["n3p"], dtype=np.float64).sum()) for r in res.results)
    )
    return out, norm3
